# revision 14
# baseline (speedup 1.0000x reference)
"""Trainium2 Bass kernel (raw Bass): per-class precision/recall sums.

Computes, for pred/gt 0-1 indicator tensors of shape [N, C]:
    intersection = sum_n pred*gt   [C]
    pred_sum     = sum_n pred      [C]
    gt_sum       = sum_n gt        [C]
    precisions   = (intersection + EPS) / (pred_sum + EPS)
    recalls      = (intersection + EPS) / (gt_sum + EPS)

Sharding: rows split across 8 NeuronCores. The host re-encodes each
core's chunk as fp8(e5m2) -- exact for 0/1 -- in 228-column blocks
    [pred(7 rows x 16 cls) | 1.0 | 0 | gt(same 7 rows) | 1.0 | 0]
staged as x[128, 592, 228] (rows per partition padded 4096 -> 592*7
with zeros; zero rows only pollute the ignored ones*ones cell). The
228 (= 0 mod 4) block width keeps every weight window 4-byte aligned
so Fast-Weight-Load stays on.

Device: one accumulating matmul per block does ALL the math:
    W = block cols 0:128   = [pred 112 | one | pad | 14 junk cols]
    R = block cols 114:227 = [gt 112 | one]
    psum[j, n] += sum_k W[k, j] * R[k, n]
  diag j=n<112   -> intersection per (r, c) slot
  col 112, j<112 -> pred sums per slot
  row 112, n<112 -> gt sums per slot
  rows 113-127, cell (112,112): junk, ignored on host.

The whole ~132 KiB/partition payload fits in SBUF, so input DMAs are
issued up front with no recycling, split across THREE DMA queues
(gpsimd SWDGE + sync/scalar HWDGE) so each SDMA engine keeps several
descriptors in flight (a single queue measured only 245 GB/s of the
358 GB/s per-core HBM limit). Small first chunks start the PE early;
small last chunks shorten the tail. PE chases per-chunk semaphores.
Epilogue: DVE copies psum -> SBUF, sync-DMA out [128, 113] fp32
partials; the host folds the 7 row-groups and sums cores in float64
(exact integer arithmetic end to end).
"""

from contextlib import ExitStack

import numpy as np

N_CORES = 8
N_ROWS, C = 4194304, 16
ROWS_PER_CORE = N_ROWS // N_CORES   # 524288
EPS = np.float32(1e-6)

P = 128
RPP = ROWS_PER_CORE // P            # 4096 rows per partition
R_GRP = 7                           # row-groups per block
D = R_GRP * C                       # 112 data cols per tensor per block
BLK_W = 228                         # 4-byte aligned block width
GT_OFF = D + 2                      # 114
M_OUT = D + 1                       # 113 meaningful out rows/cols
W_COLS = 128                        # weight window (FWL needs 128)
N_BLOCKS = 592                      # 592*7 = 4144 row slots (48 pad)

# DMA chunk sizes (blocks): small head so the PE starts early, big
# middle for descriptor efficiency, finer tail so the PE finishes right
# behind the last byte. Chunk 0 goes on the sync HWDGE queue (starts
# ~1.3 us before the SWDGE path); the rest go on gpsimd SWDGE, which
# keeps all 16 SDMA engines uniform (HWDGE makes engine 15 ~17% slower
# -- descriptor fetches contend with its AXI port -- and every chunk
# semaphore waits on the slowest engine).
#
# SDMA engine 0 (serving partitions 0-3 and 32-35) runs ~5% slower than
# its peers and takes ~1.2 us periodic stalls, so it finishes ~5 us
# late and gates every late-chunk semaphore. Relief: those 8 partitions
# carry real rows only in blocks [0, TAIL0); the tail chunks transfer
# partitions 4:32 and 36:128 only (engine 0 sits them out), with blocks
# >= TAIL0 on partitions 0-3/32-35 zeroed by an early DVE memset. Each
# tail chunk's semaphore rides on a 1-block full-partition marker DMA
# queued after the two partial DMAs on the same ring (FIFO per engine),
# so the inc fires only when every engine has drained the whole chunk.
CHUNKS = [4, 16, 32, 80, 96, 96, 96, 96, 32, 24, 12, 8]
TAIL0 = 516                         # first block absent on engine-0 partitions
N_HEAD_CHUNKS = 8                   # chunks covering blocks [0, TAIL0)
E0_PARTS = (0, 32)                  # partition range starts (4 wide each)
assert sum(CHUNKS) == N_BLOCKS
assert sum(CHUNKS[:N_HEAD_CHUNKS]) == TAIL0

ONE_E5M2 = np.uint8(0x3C)           # bit pattern of 1.0 in fp8 e5m2

_CACHE = {}
LAST_RUN = None  # BassKernelResults of the most recent run (for test harness)


def _build_nc():
    import concourse.bass as bass
    import concourse.mybir as mybir

    f32 = mybir.dt.float32
    f8 = mybir.dt.float8e5

    nc = bass.Bass()
    x_d = nc.dram_tensor("x", [P, N_BLOCKS, BLK_W], f8, kind="ExternalInput")
    out_d = nc.dram_tensor("out", [P, M_OUT], f32, kind="ExternalOutput")

    starts = np.concatenate([[0], np.cumsum(CHUNKS)])

    ctx = ExitStack()
    with ctx:
        data = ctx.enter_context(nc.sbuf_tensor("data", [P, N_BLOCKS, BLK_W], f8))
        res = ctx.enter_context(nc.sbuf_tensor("res", [P, M_OUT], f32))
        ps = ctx.enter_context(nc.psum_tensor([P, M_OUT], f32))

        csems = [
            ctx.enter_context(nc.semaphore(name=f"c{i}"))
            for i in range(len(CHUNKS))
        ]
        msem = ctx.enter_context(nc.semaphore(name="mz"))
        scrap = ctx.enter_context(nc.semaphore(name="scrap"))
        pe_sem = ctx.enter_context(nc.semaphore(name="pe"))
        dve_sem = ctx.enter_context(nc.semaphore(name="dve"))
        out_sem = ctx.enter_context(nc.semaphore(name="outd"))
        block = ctx.enter_context(nc.Block())

        @block.sync
        def _(sync):
            s, e = int(starts[0]), int(starts[1])
            sync.dma_start(
                data[:, s:e, :], x_d[:, s:e, :]
            ).then_inc(csems[0], 16)

        @block.gpsimd
        def _(gpsimd):
            for i in range(1, N_HEAD_CHUNKS):
                s, e = int(starts[i]), int(starts[i + 1])
                gpsimd.dma_start(
                    data[:, s:e, :], x_d[:, s:e, :]
                ).then_inc(csems[i], 16)
            # tail chunks skip engine-0's partitions; wait for the DVE
            # zero-fill of those partitions' tail blocks first
            gpsimd.wait_ge(msem, 1)
            for i in range(N_HEAD_CHUNKS, len(CHUNKS)):
                s, e = int(starts[i]), int(starts[i + 1])
                gpsimd.dma_start(
                    data[4:32, s:e, :], x_d[4:32, s:e, :]
                ).then_inc(scrap, 16)
                gpsimd.dma_start(
                    data[36:128, s:e, :], x_d[36:128, s:e, :]
                ).then_inc(scrap, 16)
                # full-partition 1-block marker, queued after both partial
                # DMAs on the same ring: its incs imply the chunk landed
                gpsimd.dma_start(
                    data[:, s:s + 1, :], x_d[:, s:s + 1, :]
                ).then_inc(csems[i], 16)

        @block.scalar
        def _(scalar):
            # output DMA on the scalar HWDGE queue after DVE finishes
            # the psum -> SBUF copy
            scalar.wait_ge(dve_sem, 1)
            scalar.dma_start(out_d[:, :], res[:, :]).then_inc(out_sem, 16)
            scalar.wait_ge(out_sem, 16)

        @block.tensor
        def _(tensor):
            inst = None
            for i in range(len(CHUNKS)):
                tensor.wait_ge(csems[i], 16)
                for b in range(int(starts[i]), int(starts[i + 1])):
                    inst = nc.tensor.matmul(
                        ps[:, :],
                        data[:, b, 0:W_COLS],
                        data[:, b, GT_OFF:GT_OFF + M_OUT],
                        start=(b == 0),
                        stop=(b == N_BLOCKS - 1),
                    )
            inst.then_inc(pe_sem, 1)

        @block.vector
        def _(vector):
            # zero engine-0 partitions' tail blocks (all 128 partitions
            # for DVE lane efficiency; other partitions get overwritten
            # by the tail DMAs, which wait on msem)
            vector.memset(data[:, TAIL0:, :], 0.0)
            vector.nop().then_inc(msem, 1)
            vector.wait_ge(pe_sem, 1)
            vector.tensor_copy(res[:, :], ps[:, :])
            vector.nop().then_inc(dve_sem, 1)

    return nc


def _get_nc():
    if "nc" not in _CACHE:
        _CACHE["nc"] = _build_nc()
    return _CACHE["nc"]


# Per-partition row capacity: engine-0 partitions only carry rows in
# blocks [0, TAIL0).
_CAPS = np.full(P, N_BLOCKS * R_GRP, dtype=np.int64)
for _s in E0_PARTS:
    _CAPS[_s:_s + 4] = TAIL0 * R_GRP
assert _CAPS.sum() >= ROWS_PER_CORE
_OFFS = np.concatenate([[0], np.cumsum(_CAPS)])


def _fill_rows(dst, rows_u8):
    """Distribute [ROWS_PER_CORE, C] rows into dst[P, N_BLOCKS*R_GRP, C]
    respecting per-partition capacities (zeros elsewhere)."""
    n = rows_u8.shape[0]
    for p in range(P):
        lo = min(int(_OFFS[p]), n)
        hi = min(int(_OFFS[p + 1]), n)
        if hi > lo:
            dst[p, :hi - lo] = rows_u8[lo:hi]


def _stage_core(pred_u8, gt_u8):
    """pred_u8/gt_u8: [ROWS_PER_CORE, C] uint8 0/1 -> x[P, N_BLOCKS, BLK_W]
    fp8e5m2 bit pattern (as uint8)."""
    x = np.zeros((P, N_BLOCKS, BLK_W), dtype=np.uint8)
    buf = np.zeros((P, N_BLOCKS * R_GRP, C), dtype=np.uint8)

    _fill_rows(buf, pred_u8)
    x[:, :, 0:D] = buf.reshape(P, N_BLOCKS, D) * ONE_E5M2
    x[:, :, D] = ONE_E5M2

    buf[:] = 0
    _fill_rows(buf, gt_u8)
    x[:, :, GT_OFF:GT_OFF + D] = buf.reshape(P, N_BLOCKS, D) * ONE_E5M2
    x[:, :, GT_OFF + D] = ONE_E5M2

    # ones columns must be zero where the tail is never DMA'd (engine-0
    # partitions, blocks >= TAIL0) so the SBUF memset-zeros match x.
    for _s in E0_PARTS:
        x[_s:_s + 4, TAIL0:, :] = 0
    return x


def kernel(pred, gt, **run_kwargs):
    global LAST_RUN
    import ml_dtypes
    from concourse.bass_utils import run_bass_kernel_spmd

    pred = np.asarray(pred)
    gt = np.asarray(gt)
    assert pred.shape == (N_ROWS, C) and gt.shape == (N_ROWS, C)

    pred_u8 = pred.astype(np.uint8)   # 0/1
    gt_u8 = gt.astype(np.uint8)

    in_maps = []
    for i in range(N_CORES):
        sl = slice(i * ROWS_PER_CORE, (i + 1) * ROWS_PER_CORE)
        x = _stage_core(pred_u8[sl], gt_u8[sl])
        in_maps.append({"x": x.view(ml_dtypes.float8_e5m2)})

    nc = _get_nc()
    br = run_bass_kernel_spmd(nc, in_maps, core_ids=list(range(N_CORES)),
                              **run_kwargs)
    LAST_RUN = br

    # Sum the [128, 113] per-core partials exactly, then fold the
    # 7 row-groups per class.
    T = np.zeros((P, M_OUT), dtype=np.float64)
    for r in br.results:
        T += np.asarray(r["out"], dtype=np.float64)

    diag = np.diagonal(T)[:D]                       # intersection slots
    intersection = diag.reshape(R_GRP, C).sum(axis=0).astype(np.float32)
    pred_sum = T[:D, D].reshape(R_GRP, C).sum(axis=0).astype(np.float32)
    gt_sum = T[D, :D].reshape(R_GRP, C).sum(axis=0).astype(np.float32)

    recalls = (intersection + EPS) / (gt_sum + EPS)
    precisions = (intersection + EPS) / (pred_sum + EPS)
    return (precisions, recalls, intersection, gt_sum, pred_sum)


# revision 15
# speedup vs baseline: 1.1360x; 1.1360x over previous
"""Trainium2 Bass kernel (raw Bass): per-class precision/recall sums.

Computes, for pred/gt 0-1 indicator tensors of shape [N, C]:
    intersection = sum_n pred*gt   [C]
    pred_sum     = sum_n pred      [C]
    gt_sum       = sum_n gt        [C]
    precisions   = (intersection + EPS) / (pred_sum + EPS)
    recalls      = (intersection + EPS) / (gt_sum + EPS)

Sharding: rows split across 8 NeuronCores. The host re-encodes each
core's chunk as fp8(e5m2) -- exact for 0/1 -- in 226-column blocks
    [pred(7 rows x 16 cls) | 1.0 | gt(same 7 rows) | 1.0]
staged as x[128, 592, 226] (rows per partition padded 4096 -> 592*7
with zeros; zero rows only pollute the ignored ones*ones cell).

Device: one accumulating matmul per block does ALL the math:
    W = block cols 0:128   = [pred 112 | one | 15 junk cols]
    R = block cols 113:226 = [gt 112 | one]
    psum[j, n] += sum_k W[k, j] * R[k, n]
  diag j=n<112   -> intersection per (r, c) slot
  col 112, j<112 -> pred sums per slot
  row 112, n<112 -> gt sums per slot
  rows 113-127, cell (112,112): junk, ignored on host.

The whole ~131 KiB/partition payload fits in SBUF, so input DMAs are
issued up front with no slot recycling. Chunk 0 rides the sync HWDGE
queue (starts ~1.3 us earlier than SWDGE); the rest go on gpsimd SWDGE,
which keeps all 16 SDMA engines uniform (HWDGE makes engine 15 ~17%
slower -- its descriptor fetches contend on an AXI port -- and every
chunk semaphore waits on the slowest engine). Measured: engines stream
at ~27 GB/s each (~420 GB/s aggregate, the practical fabric ceiling)
with 8-22 KB descriptors. PE chases per-chunk semaphores; the chunk
ramp starts small so the PE starts early and ends small so it finishes
right behind the last byte. Epilogue: DVE copies psum -> SBUF, scalar
HWDGE DMAs out [128, 113] fp32 partials; host folds the 7 row-groups
and sums cores in float64 (exact integer arithmetic end to end).
"""

from contextlib import ExitStack

import numpy as np

N_CORES = 8
N_ROWS, C = 4194304, 16
ROWS_PER_CORE = N_ROWS // N_CORES   # 524288
EPS = np.float32(1e-6)

P = 128
RPP = ROWS_PER_CORE // P            # 4096 rows per partition
R_GRP = 7                           # row-groups per block
D = R_GRP * C                       # 112 data cols per tensor per block
BLK_W = 2 * D + 2                   # 226
GT_OFF = D + 1                      # 113
M_OUT = D + 1                       # 113 meaningful out rows/cols
W_COLS = 128                        # weight window width
N_BLOCKS = 592                      # 592*7 = 4144 row slots (48 pad)

# DMA chunk sizes (blocks): small head so the PE starts early, big
# middle for descriptor efficiency, finer tail so the PE finishes right
# behind the last byte.
CHUNKS = [4, 12, 32, 80, 96, 96, 96, 80, 48, 24, 16, 8]
assert sum(CHUNKS) == N_BLOCKS

ONE_E5M2 = np.uint8(0x3C)           # bit pattern of 1.0 in fp8 e5m2

_CACHE = {}
LAST_RUN = None  # BassKernelResults of the most recent run (for test harness)


def _build_nc():
    import concourse.bass as bass
    import concourse.mybir as mybir

    f32 = mybir.dt.float32
    f8 = mybir.dt.float8e5

    nc = bass.Bass()
    x_d = nc.dram_tensor("x", [P, N_BLOCKS, BLK_W], f8, kind="ExternalInput")
    out_d = nc.dram_tensor("out", [P, M_OUT], f32, kind="ExternalOutput")

    starts = np.concatenate([[0], np.cumsum(CHUNKS)])

    ctx = ExitStack()
    with ctx:
        data = ctx.enter_context(nc.sbuf_tensor("data", [P, N_BLOCKS, BLK_W], f8))
        res = ctx.enter_context(nc.sbuf_tensor("res", [P, M_OUT], f32))
        ps = ctx.enter_context(nc.psum_tensor([P, M_OUT], f32))

        csems = [
            ctx.enter_context(nc.semaphore(name=f"c{i}"))
            for i in range(len(CHUNKS))
        ]
        pe_sem = ctx.enter_context(nc.semaphore(name="pe"))
        dve_sem = ctx.enter_context(nc.semaphore(name="dve"))
        out_sem = ctx.enter_context(nc.semaphore(name="outd"))
        block = ctx.enter_context(nc.Block())

        @block.sync
        def _(sync):
            s, e = int(starts[0]), int(starts[1])
            sync.dma_start(
                data[:, s:e, :], x_d[:, s:e, :]
            ).then_inc(csems[0], 16)

        @block.gpsimd
        def _(gpsimd):
            for i in range(1, len(CHUNKS)):
                s, e = int(starts[i]), int(starts[i + 1])
                gpsimd.dma_start(
                    data[:, s:e, :], x_d[:, s:e, :]
                ).then_inc(csems[i], 16)

        @block.scalar
        def _(scalar):
            # output DMA on the scalar HWDGE queue after DVE finishes
            # the psum -> SBUF copy; the end-of-program drain covers
            # completion, no explicit wait needed
            scalar.wait_ge(dve_sem, 1)
            scalar.dma_start(out_d[:, :], res[:, :]).then_inc(out_sem, 16)

        @block.tensor
        def _(tensor):
            inst = None
            for i in range(len(CHUNKS)):
                tensor.wait_ge(csems[i], 16)
                for b in range(int(starts[i]), int(starts[i + 1])):
                    inst = nc.tensor.matmul(
                        ps[:, :],
                        data[:, b, 0:W_COLS],
                        data[:, b, GT_OFF:GT_OFF + M_OUT],
                        start=(b == 0),
                        stop=(b == N_BLOCKS - 1),
                    )
            inst.then_inc(pe_sem, 1)

        @block.vector
        def _(vector):
            vector.wait_ge(pe_sem, 1)
            vector.tensor_copy(res[:, :], ps[:, :])
            vector.nop().then_inc(dve_sem, 1)

    return nc


def _get_nc():
    if "nc" not in _CACHE:
        _CACHE["nc"] = _build_nc()
    return _CACHE["nc"]


def _stage_core(pred_u8, gt_u8):
    """pred_u8/gt_u8: [ROWS_PER_CORE, C] uint8 0/1 -> x[P, N_BLOCKS, BLK_W]
    fp8e5m2 bit pattern (as uint8)."""
    x = np.empty((P, N_BLOCKS, BLK_W), dtype=np.uint8)
    pad = np.zeros((P, N_BLOCKS * R_GRP - RPP, C), dtype=np.uint8)

    pb = np.concatenate([pred_u8.reshape(P, RPP, C), pad], axis=1)
    x[:, :, 0:D] = pb.reshape(P, N_BLOCKS, D) * ONE_E5M2
    x[:, :, D] = ONE_E5M2

    gb = np.concatenate([gt_u8.reshape(P, RPP, C), pad], axis=1)
    x[:, :, GT_OFF:GT_OFF + D] = gb.reshape(P, N_BLOCKS, D) * ONE_E5M2
    x[:, :, GT_OFF + D] = ONE_E5M2
    return x


def kernel(pred, gt, **run_kwargs):
    global LAST_RUN
    import ml_dtypes
    from concourse.bass_utils import run_bass_kernel_spmd

    pred = np.asarray(pred)
    gt = np.asarray(gt)
    assert pred.shape == (N_ROWS, C) and gt.shape == (N_ROWS, C)

    pred_u8 = pred.astype(np.uint8)   # 0/1
    gt_u8 = gt.astype(np.uint8)

    in_maps = []
    for i in range(N_CORES):
        sl = slice(i * ROWS_PER_CORE, (i + 1) * ROWS_PER_CORE)
        x = _stage_core(pred_u8[sl], gt_u8[sl])
        in_maps.append({"x": x.view(ml_dtypes.float8_e5m2)})

    nc = _get_nc()
    br = run_bass_kernel_spmd(nc, in_maps, core_ids=list(range(N_CORES)),
                              **run_kwargs)
    LAST_RUN = br

    # Sum the [128, 113] per-core partials exactly, then fold the
    # 7 row-groups per class.
    T = np.zeros((P, M_OUT), dtype=np.float64)
    for r in br.results:
        T += np.asarray(r["out"], dtype=np.float64)

    diag = np.diagonal(T)[:D]                       # intersection slots
    intersection = diag.reshape(R_GRP, C).sum(axis=0).astype(np.float32)
    pred_sum = T[:D, D].reshape(R_GRP, C).sum(axis=0).astype(np.float32)
    gt_sum = T[D, :D].reshape(R_GRP, C).sum(axis=0).astype(np.float32)

    recalls = (intersection + EPS) / (gt_sum + EPS)
    precisions = (intersection + EPS) / (pred_sum + EPS)
    return (precisions, recalls, intersection, gt_sum, pred_sum)


# revision 16
# speedup vs baseline: 1.1720x; 1.0316x over previous
"""Trainium2 Bass kernel (raw Bass): per-class precision/recall sums.

Computes, for pred/gt 0-1 indicator tensors of shape [N, C]:
    intersection = sum_n pred*gt   [C]
    pred_sum     = sum_n pred      [C]
    gt_sum       = sum_n gt        [C]
    precisions   = (intersection + EPS) / (pred_sum + EPS)
    recalls      = (intersection + EPS) / (gt_sum + EPS)

Sharding: rows split across 8 NeuronCores. The host re-encodes each
core's chunk as fp8(e5m2) -- exact for 0/1 -- in 226-column blocks
    [pred(7 rows x 16 cls) | 1.0 | gt(same 7 rows) | 1.0]
staged as x[128, 592, 226] (rows per partition padded 4096 -> 592*7
with zeros; zero rows only pollute the ignored ones*ones cell).

Device: one accumulating matmul per block does ALL the math:
    W = block cols 0:128   = [pred 112 | one | 15 junk cols]
    R = block cols 113:226 = [gt 112 | one]
    psum[j, n] += sum_k W[k, j] * R[k, n]
  diag j=n<112   -> intersection per (r, c) slot
  col 112, j<112 -> pred sums per slot
  row 112, n<112 -> gt sums per slot
  rows 113-127, cell (112,112): junk, ignored on host.

The whole ~131 KiB/partition payload fits in SBUF, so input DMAs are
issued up front with no slot recycling. Chunk 0 rides the sync HWDGE
queue (starts ~1.3 us earlier than SWDGE); the rest go on gpsimd SWDGE,
which keeps all 16 SDMA engines uniform (HWDGE makes engine 15 ~17%
slower -- its descriptor fetches contend on an AXI port -- and every
chunk semaphore waits on the slowest engine). Measured: engines stream
at ~27 GB/s each (~420 GB/s aggregate, the practical fabric ceiling)
with 8-22 KB descriptors. PE chases per-chunk semaphores; the chunk
ramp starts small so the PE starts early and ends small so it finishes
right behind the last byte. Epilogue: DVE copies psum -> SBUF, scalar
HWDGE DMAs out [128, 113] fp32 partials; host folds the 7 row-groups
and sums cores in float64 (exact integer arithmetic end to end).
"""

from contextlib import ExitStack

import numpy as np

N_CORES = 8
N_ROWS, C = 4194304, 16
ROWS_PER_CORE = N_ROWS // N_CORES   # 524288
EPS = np.float32(1e-6)

P = 128
RPP = ROWS_PER_CORE // P            # 4096 rows per partition
R_GRP = 7                           # row-groups per block
D = R_GRP * C                       # 112 data cols per tensor per block
BLK_W = 2 * D + 2                   # 226
GT_OFF = D + 1                      # 113
M_OUT = D + 1                       # 113 meaningful out rows/cols
W_COLS = 128                        # weight window width
N_BLOCKS = 592                      # 592*7 = 4144 row slots (48 pad)

# DMA chunk sizes (blocks): small head so the PE starts early, big
# middle for descriptor efficiency, finer tail so the PE finishes right
# behind the last byte.
CHUNKS = [4, 20, 80, 96, 96, 96, 80, 48, 32, 16, 8, 8, 8]
assert sum(CHUNKS) == N_BLOCKS

ONE_E5M2 = np.uint8(0x3C)           # bit pattern of 1.0 in fp8 e5m2

_CACHE = {}
LAST_RUN = None  # BassKernelResults of the most recent run (for test harness)


def _build_nc():
    import concourse.bass as bass
    import concourse.mybir as mybir

    f32 = mybir.dt.float32
    f8 = mybir.dt.float8e5

    nc = bass.Bass()
    x_d = nc.dram_tensor("x", [P, N_BLOCKS, BLK_W], f8, kind="ExternalInput")
    out_d = nc.dram_tensor("out", [P, M_OUT], f32, kind="ExternalOutput")

    starts = np.concatenate([[0], np.cumsum(CHUNKS)])

    ctx = ExitStack()
    with ctx:
        data = ctx.enter_context(nc.sbuf_tensor("data", [P, N_BLOCKS, BLK_W], f8))
        res = ctx.enter_context(nc.sbuf_tensor("res", [P, M_OUT], f32))
        ps = ctx.enter_context(nc.psum_tensor([P, M_OUT], f32))

        csems = [
            ctx.enter_context(nc.semaphore(name=f"c{i}"))
            for i in range(len(CHUNKS))
        ]
        pe_sem = ctx.enter_context(nc.semaphore(name="pe"))
        dve_sem = ctx.enter_context(nc.semaphore(name="dve"))
        out_sem = ctx.enter_context(nc.semaphore(name="outd"))
        block = ctx.enter_context(nc.Block())

        @block.sync
        def _(sync):
            s, e = int(starts[0]), int(starts[1])
            sync.dma_start(
                data[:, s:e, :], x_d[:, s:e, :]
            ).then_inc(csems[0], 16)

        @block.gpsimd
        def _(gpsimd):
            for i in range(1, len(CHUNKS)):
                s, e = int(starts[i]), int(starts[i + 1])
                gpsimd.dma_start(
                    data[:, s:e, :], x_d[:, s:e, :]
                ).then_inc(csems[i], 16)

        @block.scalar
        def _(scalar):
            # output DMA on the scalar HWDGE queue after DVE finishes
            # the psum -> SBUF copy; the end-of-program drain covers
            # completion, no explicit wait needed
            scalar.wait_ge(dve_sem, 1)
            scalar.dma_start(out_d[:, :], res[:, :]).then_inc(out_sem, 16)

        @block.tensor
        def _(tensor):
            inst = None
            for i in range(len(CHUNKS)):
                tensor.wait_ge(csems[i], 16)
                for b in range(int(starts[i]), int(starts[i + 1])):
                    inst = nc.tensor.matmul(
                        ps[:, :],
                        data[:, b, 0:W_COLS],
                        data[:, b, GT_OFF:GT_OFF + M_OUT],
                        start=(b == 0),
                        stop=(b == N_BLOCKS - 1),
                    )
            inst.then_inc(pe_sem, 1)

        @block.vector
        def _(vector):
            vector.wait_ge(pe_sem, 1)
            vector.tensor_copy(res[:, :], ps[:, :])
            vector.nop().then_inc(dve_sem, 1)

    return nc


def _get_nc():
    if "nc" not in _CACHE:
        _CACHE["nc"] = _build_nc()
    return _CACHE["nc"]


def _stage_core(pred_u8, gt_u8):
    """pred_u8/gt_u8: [ROWS_PER_CORE, C] uint8 0/1 -> x[P, N_BLOCKS, BLK_W]
    fp8e5m2 bit pattern (as uint8)."""
    x = np.empty((P, N_BLOCKS, BLK_W), dtype=np.uint8)
    pad = np.zeros((P, N_BLOCKS * R_GRP - RPP, C), dtype=np.uint8)

    pb = np.concatenate([pred_u8.reshape(P, RPP, C), pad], axis=1)
    x[:, :, 0:D] = pb.reshape(P, N_BLOCKS, D) * ONE_E5M2
    x[:, :, D] = ONE_E5M2

    gb = np.concatenate([gt_u8.reshape(P, RPP, C), pad], axis=1)
    x[:, :, GT_OFF:GT_OFF + D] = gb.reshape(P, N_BLOCKS, D) * ONE_E5M2
    x[:, :, GT_OFF + D] = ONE_E5M2
    return x


def kernel(pred, gt, **run_kwargs):
    global LAST_RUN
    import ml_dtypes
    from concourse.bass_utils import run_bass_kernel_spmd

    pred = np.asarray(pred)
    gt = np.asarray(gt)
    assert pred.shape == (N_ROWS, C) and gt.shape == (N_ROWS, C)

    pred_u8 = pred.astype(np.uint8)   # 0/1
    gt_u8 = gt.astype(np.uint8)

    in_maps = []
    for i in range(N_CORES):
        sl = slice(i * ROWS_PER_CORE, (i + 1) * ROWS_PER_CORE)
        x = _stage_core(pred_u8[sl], gt_u8[sl])
        in_maps.append({"x": x.view(ml_dtypes.float8_e5m2)})

    nc = _get_nc()
    br = run_bass_kernel_spmd(nc, in_maps, core_ids=list(range(N_CORES)),
                              **run_kwargs)
    LAST_RUN = br

    # Sum the [128, 113] per-core partials exactly, then fold the
    # 7 row-groups per class.
    T = np.zeros((P, M_OUT), dtype=np.float64)
    for r in br.results:
        T += np.asarray(r["out"], dtype=np.float64)

    diag = np.diagonal(T)[:D]                       # intersection slots
    intersection = diag.reshape(R_GRP, C).sum(axis=0).astype(np.float32)
    pred_sum = T[:D, D].reshape(R_GRP, C).sum(axis=0).astype(np.float32)
    gt_sum = T[D, :D].reshape(R_GRP, C).sum(axis=0).astype(np.float32)

    recalls = (intersection + EPS) / (gt_sum + EPS)
    precisions = (intersection + EPS) / (pred_sum + EPS)
    return (precisions, recalls, intersection, gt_sum, pred_sum)


# revision 20
# speedup vs baseline: 1.2302x; 1.0497x over previous
"""Trainium2 Bass kernel (raw Bass): per-class precision/recall sums.

Computes, for pred/gt 0-1 indicator tensors of shape [N, C]:
    intersection = sum_n pred*gt   [C]
    pred_sum     = sum_n pred      [C]
    gt_sum       = sum_n gt        [C]
    precisions   = (intersection + EPS) / (pred_sum + EPS)
    recalls      = (intersection + EPS) / (gt_sum + EPS)

Sharding: rows split across 8 NeuronCores. The host re-encodes each
core's chunk as fp8(e5m2) -- exact for 0/1 -- in 226-column blocks
    [pred(7 rows x 16 cls) | 1.0 | gt(same 7 rows) | 1.0]
staged as x[128, 592, 226] (rows per partition padded 4096 -> 592*7
with zeros; zero rows only pollute the ignored ones*ones cell).

Device: one accumulating matmul per block does ALL the math:
    W = block cols 0:128   = [pred 112 | one | 15 junk cols]
    R = block cols 113:226 = [gt 112 | one]
    psum[j, n] += sum_k W[k, j] * R[k, n]
  diag j=n<112   -> intersection per (r, c) slot
  col 112, j<112 -> pred sums per slot
  row 112, n<112 -> gt sums per slot
  rows 113-127, cell (112,112): junk, ignored on host.

The whole ~131 KiB/partition payload fits in SBUF, so input DMAs are
issued up front with no slot recycling. Chunk 0 rides the sync HWDGE
queue (starts ~1.3 us earlier than SWDGE); the rest go on gpsimd SWDGE,
which keeps all 16 SDMA engines uniform (HWDGE makes engine 15 ~17%
slower -- its descriptor fetches contend on an AXI port -- and every
chunk semaphore waits on the slowest engine). Measured: engines stream
at ~27 GB/s each (~420 GB/s aggregate, the practical fabric ceiling)
with 8-22 KB descriptors. PE chases per-chunk semaphores; the chunk
ramp starts small so the PE starts early and ends small so it finishes
right behind the last byte. Epilogue: DVE copies psum -> SBUF, scalar
HWDGE DMAs out [128, 113] fp32 partials; host folds the 7 row-groups
and sums cores in float64 (exact integer arithmetic end to end).
"""

from contextlib import ExitStack

import numpy as np

N_CORES = 8
N_ROWS, C = 4194304, 16
ROWS_PER_CORE = N_ROWS // N_CORES   # 524288
EPS = np.float32(1e-6)

P = 128
RPP = ROWS_PER_CORE // P            # 4096 rows per partition
R_GRP = 7                           # row-groups per block
D = R_GRP * C                       # 112 data cols per tensor per block
BLK_W = 2 * D + 2                   # 226
GT_OFF = D + 1                      # 113
M_OUT = D + 1                       # 113 meaningful out rows/cols
W_COLS = 128                        # weight window width
N_BLOCKS = 586                      # 585 full blocks + 1 block with the
                                    # last row (585*7 + 1 = 4096, exact)

# DMA chunk sizes (blocks): small head so the PE starts early, big
# middle for descriptor efficiency, finer tail so the PE finishes right
# behind the last byte.
CHUNKS = [4, 20, 80, 96, 96, 96, 80, 48, 26, 16, 8, 8, 8]
assert sum(CHUNKS) == N_BLOCKS

ONE_E5M2 = np.uint8(0x3C)           # bit pattern of 1.0 in fp8 e5m2

_CACHE = {}
LAST_RUN = None  # BassKernelResults of the most recent run (for test harness)


def _build_nc():
    import concourse.bass as bass
    import concourse.mybir as mybir

    f32 = mybir.dt.float32
    f8 = mybir.dt.float8e5

    nc = bass.Bass()
    x_d = nc.dram_tensor("x", [P, N_BLOCKS, BLK_W], f8, kind="ExternalInput")
    out_d = nc.dram_tensor("out", [P, M_OUT], f32, kind="ExternalOutput")

    starts = np.concatenate([[0], np.cumsum(CHUNKS)])

    ctx = ExitStack()
    with ctx:
        data = ctx.enter_context(nc.sbuf_tensor("data", [P, N_BLOCKS, BLK_W], f8))
        res = ctx.enter_context(nc.sbuf_tensor("res", [P, M_OUT], f32))
        ps = ctx.enter_context(nc.psum_tensor([P, M_OUT], f32))

        csems = [
            ctx.enter_context(nc.semaphore(name=f"c{i}"))
            for i in range(len(CHUNKS))
        ]
        pe_sem = ctx.enter_context(nc.semaphore(name="pe"))
        cp_sem = ctx.enter_context(nc.semaphore(name="cp"))
        out_sem = ctx.enter_context(nc.semaphore(name="outd"))
        block = ctx.enter_context(nc.Block())

        @block.sync
        def _(sync):
            s, e = int(starts[0]), int(starts[1])
            sync.dma_start(
                data[:, s:e, :], x_d[:, s:e, :]
            ).then_inc(csems[0], 16)

        @block.gpsimd
        def _(gpsimd):
            for i in range(1, len(CHUNKS)):
                s, e = int(starts[i]), int(starts[i + 1])
                gpsimd.dma_start(
                    data[:, s:e, :], x_d[:, s:e, :]
                ).then_inc(csems[i], 16)

        @block.scalar
        def _(scalar):
            # whole epilogue on ScalarE (it can read PSUM): copy psum ->
            # SBUF, then DMA out on its HWDGE queue. The self-semaphore
            # between copy and dma_start orders the HWDGE descriptor
            # generation after the copy's datapath completion. The
            # end-of-program drain covers output completion, no final
            # wait needed.
            scalar.wait_ge(pe_sem, 1)
            scalar.copy(res[:, :], ps[:, :]).then_inc(cp_sem, 1)
            scalar.wait_ge(cp_sem, 1)
            scalar.dma_start(out_d[:, :], res[:, :]).then_inc(out_sem, 16)

        @block.tensor
        def _(tensor):
            inst = None
            for i in range(len(CHUNKS)):
                tensor.wait_ge(csems[i], 16)
                for b in range(int(starts[i]), int(starts[i + 1])):
                    inst = nc.tensor.matmul(
                        ps[:, :],
                        data[:, b, 0:W_COLS],
                        data[:, b, GT_OFF:GT_OFF + M_OUT],
                        start=(b == 0),
                        stop=(b == N_BLOCKS - 1),
                    )
            inst.then_inc(pe_sem, 1)

    return nc


def _get_nc():
    if "nc" not in _CACHE:
        _CACHE["nc"] = _build_nc()
    return _CACHE["nc"]


def _stage_core(pred_u8, gt_u8):
    """pred_u8/gt_u8: [ROWS_PER_CORE, C] uint8 0/1 -> x[P, N_BLOCKS, BLK_W]
    fp8e5m2 bit pattern (as uint8)."""
    x = np.empty((P, N_BLOCKS, BLK_W), dtype=np.uint8)
    pad = np.zeros((P, N_BLOCKS * R_GRP - RPP, C), dtype=np.uint8)

    pb = np.concatenate([pred_u8.reshape(P, RPP, C), pad], axis=1)
    x[:, :, 0:D] = pb.reshape(P, N_BLOCKS, D) * ONE_E5M2
    x[:, :, D] = ONE_E5M2

    gb = np.concatenate([gt_u8.reshape(P, RPP, C), pad], axis=1)
    x[:, :, GT_OFF:GT_OFF + D] = gb.reshape(P, N_BLOCKS, D) * ONE_E5M2
    x[:, :, GT_OFF + D] = ONE_E5M2
    return x


def kernel(pred, gt, **run_kwargs):
    global LAST_RUN
    import ml_dtypes
    from concourse.bass_utils import run_bass_kernel_spmd

    pred = np.asarray(pred)
    gt = np.asarray(gt)
    assert pred.shape == (N_ROWS, C) and gt.shape == (N_ROWS, C)

    pred_u8 = pred.astype(np.uint8)   # 0/1
    gt_u8 = gt.astype(np.uint8)

    in_maps = []
    for i in range(N_CORES):
        sl = slice(i * ROWS_PER_CORE, (i + 1) * ROWS_PER_CORE)
        x = _stage_core(pred_u8[sl], gt_u8[sl])
        in_maps.append({"x": x.view(ml_dtypes.float8_e5m2)})

    nc = _get_nc()
    br = run_bass_kernel_spmd(nc, in_maps, core_ids=list(range(N_CORES)),
                              **run_kwargs)
    LAST_RUN = br

    # Sum the [128, 113] per-core partials exactly, then fold the
    # 7 row-groups per class.
    T = np.zeros((P, M_OUT), dtype=np.float64)
    for r in br.results:
        T += np.asarray(r["out"], dtype=np.float64)

    diag = np.diagonal(T)[:D]                       # intersection slots
    intersection = diag.reshape(R_GRP, C).sum(axis=0).astype(np.float32)
    pred_sum = T[:D, D].reshape(R_GRP, C).sum(axis=0).astype(np.float32)
    gt_sum = T[D, :D].reshape(R_GRP, C).sum(axis=0).astype(np.float32)

    recalls = (intersection + EPS) / (gt_sum + EPS)
    precisions = (intersection + EPS) / (pred_sum + EPS)
    return (precisions, recalls, intersection, gt_sum, pred_sum)
